# revision 9
# baseline (speedup 1.0000x reference)
"""Training-mode BatchNorm2d over x(64,256,56,56) f32 on 8 trn2 NeuronCores.

Sharding: channel-parallel (32 channels per core) — each core owns complete
per-channel reductions, so no cross-core collectives are needed.

The 2e-2 rel-err budget admits a bf16 HBM data path: the host converts x to
bf16 (max rounding error ~2^-9 of value), the device reads bf16, computes
stats in f32, normalizes, and writes bf16 back; the host converts the output
to f32. HBM traffic per core halves to 12.85 MB read + 12.85 MB write
(~63-67us at the measured per-core aggregate DMA rate) — the floor this
kernel is built around.

Layout: per core 4 channel-blocks of 8 channels; each block is four
quarter-tiles [128p, 3136] bf16 (partition p = b_lo*8 + cc, quarter = b_hi),
so 16 loads + 16 stores of 800KB. All 16 quarters stay resident in SBUF
(12.25 MB) between the stats pass and the normalize pass (minimal 2x HBM
traffic). 8-channel blocks mean the scalar tail runs only 4 times, halving
its per-element cost vs 4-channel blocks.

Per block the two streaming moments (sum, sum of squares) are split so no
engine exceeds the ~16.8us/block DMA pace:
 - quarters 2,3: DVE bn_stats (a 1x pass yields per-partition mean AND var
   in one go, ~7.4us); var is converted to E[x^2] in place.
 - quarters 0,1: ScalarE Square activation with accum_out -> per-partition
   sum(x^2) (~6.8us); per-channel sum(x) on the TensorEngine: 7 matmuls
   per quarter of x-chunks [128, 448] (moving, bf16) against a
   (1/16)-scaled channel-indicator (stationary, bf16; 1/16 is exact),
   PSUM-accumulated into [8, 448] and folded by one DVE reduce_sum.
 - both per-partition stat sets are packed into one [128, 3] tile and
   reduced per-channel by a single tiny f32 matmul.
 - scalar tail (var, rsqrt, A=gamma*rstd, B=beta-mean*A) on DVE (its small
   ops are ~3x cheaper than ACT's); Sqrt on ACT (DVE has none); (A, B)
   broadcast to 128 partitions by a tiny PE matmul.
 - normalize x*A + B in place: quarters 0,1 on ACT (Identity with
   per-partition scale/bias APs, ~2.8us each), quarters 2,3 on DVE
   (tensor_scalar, 4x bf16 mode, ~1.25us each); the last block normalizes
   entirely on DVE to shorten the drain.

The tail of block k is emitted one block late (after block k+1's stats),
so the cross-engine chain latency hides under the next block's streaming
work. Input DMAs ride the SP HWDGE ring (no waits ever land there, so all
16 loads stream back-to-back); output DMAs ride the ACT HWDGE ring, pushed
right after the ACT norms so store waits barely stall the ACT stream.
"""

from contextlib import ExitStack

import ml_dtypes
import numpy as np

import concourse.bass as bass
import concourse.tile as tile
from concourse import bacc, mybir
from concourse.bass_utils import run_bass_kernel_spmd

F32 = mybir.dt.float32
BF16 = mybir.dt.bfloat16
NP_BF16 = np.dtype(ml_dtypes.bfloat16)

B, C, H, W = 64, 256, 56, 56
HW = H * W  # 3136
N_CORES = 8
C_LOC = C // N_CORES  # 32 channels per core
CBLK = 8  # channels per block
N_BLOCKS = C_LOC // CBLK  # 4 blocks per core
BL = 128 // CBLK  # 16 b_lo values packed in the partition dim
BH = B // BL  # 4 quarter-tiles per block (b_hi)
N_TILE = N_BLOCKS * BH  # 16 tiles per core
SUB = 448  # bn_stats subgroup / PE chunk width (3136 = 7*448, <= 512)
NSUB = HW // SUB  # 7
HALF_N = 2 * HW  # elems per partition in either stat half = 6272
EPS = 1e-5

_NC_CACHE = {}


def _build_nc(nbufs=16):
    # Bacc (not plain Bass): its finalize() runs generate_event_semaphores,
    # which splits multi-sem waits — TRN2 instructions carry at most one.
    nc = bacc.Bacc()
    x = nc.dram_tensor("x", [N_TILE, 128, HW], BF16, kind="ExternalInput")
    y = nc.dram_tensor("y", [N_TILE, 128, HW], BF16, kind="ExternalOutput")
    gamma = nc.dram_tensor("gamma", [CBLK, N_BLOCKS], F32, kind="ExternalInput")
    beta = nc.dram_tensor("beta", [CBLK, N_BLOCKS], F32, kind="ExternalInput")
    sel8b = nc.dram_tensor("sel8b", [128, CBLK], BF16, kind="ExternalInput")
    sel8f = nc.dram_tensor("sel8f", [128, CBLK], F32, kind="ExternalInput")
    selT = nc.dram_tensor("selT", [CBLK, 128], F32, kind="ExternalInput")

    AF = mybir.ActivationFunctionType
    OP = mybir.AluOpType

    with ExitStack() as ctx:
        tc = ctx.enter_context(tile.TileContext(nc))
        xpool = ctx.enter_context(tc.tile_pool(name="xdata", bufs=nbufs))
        qpool = ctx.enter_context(tc.tile_pool(name="sqscr", bufs=2))
        spool = ctx.enter_context(tc.tile_pool(name="stats", bufs=4))
        cpool = ctx.enter_context(tc.tile_pool(name="const", bufs=1))
        ppool = ctx.enter_context(tc.tile_pool(name="psum", bufs=2, space="PSUM"))

        sel8b_t = cpool.tile([128, CBLK], BF16)
        nc.gpsimd.dma_start(out=sel8b_t, in_=sel8b[:, :])
        sel8f_t = cpool.tile([128, CBLK], F32)
        nc.gpsimd.dma_start(out=sel8f_t, in_=sel8f[:, :])
        selT_t = cpool.tile([CBLK, 128], F32)
        nc.gpsimd.dma_start(out=selT_t, in_=selT[:, :])
        gam_t = cpool.tile([CBLK, N_BLOCKS], F32)
        nc.gpsimd.dma_start(out=gam_t, in_=gamma[:, :])
        bet_t = cpool.tile([CBLK, N_BLOCKS], F32)
        nc.gpsimd.dma_start(out=bet_t, in_=beta[:, :])
        eps_t = cpool.tile([CBLK, 1], F32)
        nc.vector.memset(eps_t, EPS)

        def stats_phase(blk):
            xts = []
            pack = spool.tile([128, 4], F32)
            stats = spool.tile([128, 2, NSUB, 6], F32)
            psum_s = ppool.tile([CBLK, SUB], F32, tag="ps")
            # quarters 0,1: ACT sum(x^2) + PE per-channel sum(x)/16
            for q in range(2):
                xt = xpool.tile([128, HW], BF16, tag="x")
                nc.sync.dma_start(out=xt, in_=x[blk * BH + q, :, :])
                xts.append(xt)
                scr = qpool.tile([128, HW], BF16, tag="scr")
                nc.scalar.activation(
                    scr, xt, AF.Square, accum_out=pack[:, 2 + q : 3 + q]
                )
                xv = xt.rearrange("p (s f) -> p s f", f=SUB)
                for s in range(NSUB):
                    nc.tensor.matmul(
                        psum_s,
                        sel8b_t,
                        xv[:, s, :],
                        start=(q == 0 and s == 0),
                        stop=(q == 1 and s == NSUB - 1),
                    )
            # quarters 2,3: DVE bn_stats
            for q in range(2, 4):
                xt = xpool.tile([128, HW], BF16, tag="x")
                nc.sync.dma_start(out=xt, in_=x[blk * BH + q, :, :])
                xts.append(xt)
                xv = xt.rearrange("p (s f) -> p s f", f=SUB)
                for s in range(NSUB):
                    nc.vector.bn_stats(out=stats[:, q - 2, s, :], in_=xv[:, s, :])

            # DVE: per-partition [mean, var] over q2+q3 -> [mean, E[x^2]]
            nc.vector.bn_aggr(out=pack[:, 0:2], in_=stats)
            m2 = spool.tile([128, 1], F32)
            nc.vector.tensor_mul(m2, pack[:, 0:1], pack[:, 0:1])
            nc.vector.tensor_add(pack[:, 1:2], pack[:, 1:2], m2)

            # PE: per-channel [mean_q23, E2_q23, sumsq_q0, sumsq_q1] / 16
            pq = ppool.tile([CBLK, 4], F32, tag="pq")
            nc.tensor.matmul(pq, sel8f_t, pack, start=True, stop=True)
            return xts, psum_s, pq

        def norm_phase(blk, xts, psum_s, pq):
            """Fold + scalar tail + normalize + stores. Emitted one block
            late so the cross-engine round-trips hide under the next
            block's streaming work."""
            last = blk == N_BLOCKS - 1
            # fold PE sums: s4 = sum(x_q01)/16 per channel
            s4 = spool.tile([CBLK, 1], F32)
            nc.vector.reduce_sum(s4, psum_s, axis=mybir.AxisListType.X)
            mh1 = spool.tile([CBLK, 1], F32)
            nc.vector.tensor_scalar_mul(mh1, s4, 1.0 / HALF_N)
            # mean = (mean_q23 + mean_q01)/2
            mean = spool.tile([CBLK, 1], F32)
            nc.vector.tensor_scalar(
                out=mean, in0=pq[:, 0:1], scalar1=mh1, scalar2=0.5,
                op0=OP.add, op1=OP.mult,
            )
            # E[x^2] = (E2_q23 + (sumsq_q0 + sumsq_q1)/6272)/2
            e1 = spool.tile([CBLK, 1], F32)
            nc.vector.tensor_scalar_mul(e1, pq[:, 2:3], 1.0 / HALF_N)
            e2 = spool.tile([CBLK, 1], F32)
            nc.vector.tensor_scalar(
                out=e2, in0=pq[:, 3:4], scalar1=1.0 / HALF_N, scalar2=e1,
                op0=OP.mult, op1=OP.add,
            )
            ex2 = spool.tile([CBLK, 1], F32)
            nc.vector.tensor_scalar(
                out=ex2, in0=pq[:, 1:2], scalar1=e2, scalar2=0.5,
                op0=OP.add, op1=OP.mult,
            )
            m2b = spool.tile([CBLK, 1], F32)
            nc.vector.tensor_mul(m2b, mean, mean)
            var = spool.tile([CBLK, 1], F32)
            nc.vector.tensor_sub(var, ex2, m2b)
            std = spool.tile([CBLK, 1], F32)
            nc.scalar.activation(std, var, AF.Sqrt, bias=eps_t)
            rstd = spool.tile([CBLK, 1], F32)
            nc.vector.reciprocal(rstd, std)
            # A = gamma*rstd, B = beta - mean*A
            ab8 = spool.tile([CBLK, 2], F32)
            nc.vector.tensor_mul(ab8[:, 0:1], rstd, gam_t[:, blk : blk + 1])
            t4 = spool.tile([CBLK, 1], F32)
            nc.vector.tensor_mul(t4, mean, ab8[:, 0:1])
            nc.vector.tensor_sub(ab8[:, 1:2], bet_t[:, blk : blk + 1], t4)

            # broadcast (A, B) to all 128 partitions via PE matmul
            ps2 = ppool.tile([128, 2], F32, tag="pb")
            nc.tensor.matmul(ps2, selT_t, ab8, start=True, stop=True)
            ab = spool.tile([128, 2], F32)
            nc.vector.tensor_copy(ab, ps2)

            # normalize + store; stores ride the ACT HWDGE ring, pushed
            # right after the ACT norms so the waits barely stall ACT
            def norm_dve(q):
                nc.vector.tensor_scalar(
                    out=xts[q], in0=xts[q], scalar1=ab[:, 0:1],
                    scalar2=ab[:, 1:2], op0=OP.mult, op1=OP.add,
                )

            if last:
                for q in range(4):
                    norm_dve(q)
                    nc.scalar.dma_start(out=y[blk * BH + q, :, :], in_=xts[q])
            else:
                for q in range(2):
                    norm_dve(q + 2)
                for q in range(2):
                    nc.scalar.activation(
                        xts[q], xts[q], AF.Identity,
                        bias=ab[:, 1:2], scale=ab[:, 0:1],
                    )
                    nc.scalar.dma_start(out=y[blk * BH + q, :, :], in_=xts[q])
                for q in range(2, 4):
                    nc.scalar.dma_start(out=y[blk * BH + q, :, :], in_=xts[q])

        # One-block-deep software pipeline over the emission order.
        prev = None
        for blk in range(N_BLOCKS):
            cur = stats_phase(blk)
            if blk == 0:
                norm_phase(blk, *cur)
            else:
                if prev is not None:
                    norm_phase(prev[0], *prev[1])
                prev = (blk, cur)
        if prev is not None:
            norm_phase(prev[0], *prev[1])
    nc.finalize()
    return nc


def get_nc(nbufs=16):
    if nbufs not in _NC_CACHE:
        _NC_CACHE[nbufs] = _build_nc(nbufs)
    return _NC_CACHE[nbufs]


def _sel_matrices():
    # the 1/16 channel-indicator: reduce-matmuls on per-partition values
    # yield (sum over the channel's 16 partitions)/16
    sel = np.zeros((128, CBLK), dtype=np.float32)
    sel[np.arange(128), np.arange(128) % CBLK] = 1.0 / BL
    selT = np.zeros((CBLK, 128), dtype=np.float32)
    selT[np.arange(128) % CBLK, np.arange(128)] = 1.0
    return sel, selT


def pack_inputs(x, gamma, beta):
    """Full f32 inputs -> list of per-core in_maps (bf16 device layout)."""
    x16 = np.asarray(x, dtype=np.float32).astype(NP_BF16)
    gamma = np.asarray(gamma, dtype=np.float32)
    beta = np.asarray(beta, dtype=np.float32)
    # [b_hi, b_lo, core, blk, cc, hw] -> [core, blk, b_hi, b_lo, cc, hw]
    xr = np.ascontiguousarray(
        x16.reshape(BH, BL, N_CORES, N_BLOCKS, CBLK, HW).transpose(2, 3, 0, 1, 4, 5)
    )
    g = gamma.reshape(N_CORES, N_BLOCKS, CBLK)
    bt = beta.reshape(N_CORES, N_BLOCKS, CBLK)
    sel, selT = _sel_matrices()
    sel8b = sel.astype(NP_BF16)  # 1/16 is exact in bf16
    in_maps = []
    for i in range(N_CORES):
        in_maps.append(
            {
                "x": xr[i].reshape(N_TILE, 128, HW),
                "gamma": np.ascontiguousarray(g[i].T),
                "beta": np.ascontiguousarray(bt[i].T),
                "sel8b": sel8b,
                "sel8f": sel,
                "selT": selT,
            }
        )
    return in_maps


def unpack_outputs(per_core_y):
    """List of per-core y (bf16 device layout) -> full f32 (64,256,56,56)."""
    ys = np.stack(per_core_y)  # [core, blk*b_hi, 128, hw] bf16
    out = (
        ys.reshape(N_CORES, N_BLOCKS, BH, BL, CBLK, HW)
        .transpose(2, 3, 0, 1, 4, 5)
        .astype(np.float32)
        .reshape(B, C, H, W)
    )
    return out


def run(inputs, trace=False, nbufs=16):
    """Returns (full_output, BassKernelResults)."""
    nc = get_nc(nbufs)
    in_maps = pack_inputs(inputs["x"], inputs["gamma"], inputs["beta"])
    res = run_bass_kernel_spmd(nc, in_maps, list(range(N_CORES)), trace=trace)
    out = unpack_outputs([r["y"] for r in res.results])
    return out, res


def kernel(**inputs):
    out, _ = run(inputs)
    return out


# revision 11
# speedup vs baseline: 1.0660x; 1.0660x over previous
"""Training-mode BatchNorm2d over x(64,256,56,56) f32 on 8 trn2 NeuronCores.

Sharding: channel-parallel (32 channels per core) — each core owns complete
per-channel reductions, so no cross-core collectives are needed.

The 2e-2 rel-err budget admits a bf16 HBM data path: the host converts x to
bf16 (max rounding error ~2^-9 of value), the device reads bf16, computes
stats in f32, normalizes, and writes bf16 back; the host converts the output
to f32. HBM traffic per core halves to 12.85 MB read + 12.85 MB write
(~63-67us at the measured per-core aggregate DMA rate) — the floor this
kernel is built around.

Layout: per core 4 channel-blocks of 8 channels; each block is four
quarter-tiles [128p, 3136] bf16 (partition p = b_lo*8 + cc, quarter = b_hi),
so 16 loads + 16 stores of 800KB. All 16 quarters stay resident in SBUF
(12.25 MB) between the stats pass and the normalize pass (minimal 2x HBM
traffic). 8-channel blocks mean the scalar tail runs only 4 times, halving
its per-element cost vs 4-channel blocks.

Per block the two streaming moments (sum, sum of squares) are split so no
engine exceeds the ~16.8us/block DMA pace:
 - quarters 2,3: DVE bn_stats (a 1x pass yields per-partition mean AND var
   in one go, ~7.4us); var is converted to E[x^2] in place.
 - quarters 0,1: ScalarE Square activation with accum_out -> per-partition
   sum(x^2) (~6.8us); per-channel sum(x) on the TensorEngine: 7 matmuls
   per quarter of x-chunks [128, 448] (moving, bf16) against a
   (1/16)-scaled channel-indicator (stationary, bf16; 1/16 is exact),
   PSUM-accumulated into [8, 448] and folded by one DVE reduce_sum.
 - both per-partition stat sets are packed into one [128, 3] tile and
   reduced per-channel by a single tiny f32 matmul.
 - scalar tail (var, rsqrt, A=gamma*rstd, B=beta-mean*A) on DVE (its small
   ops are ~3x cheaper than ACT's); Sqrt on ACT (DVE has none); (A, B)
   broadcast to 128 partitions by a tiny PE matmul.
 - normalize x*A + B in place: quarters 0,1 on ACT (Identity with
   per-partition scale/bias APs, ~2.8us each), quarters 2,3 on DVE
   (tensor_scalar, 4x bf16 mode, ~1.25us each); the last block normalizes
   entirely on DVE to shorten the drain.

The tail of block k is emitted one block late (after block k+1's stats),
so the cross-engine chain latency hides under the next block's streaming
work. Input DMAs ride the SP HWDGE ring (no waits ever land there, so all
16 loads stream back-to-back); output DMAs ride the ACT HWDGE ring, pushed
right after the ACT norms so store waits barely stall the ACT stream.
"""

from contextlib import ExitStack

import ml_dtypes
import numpy as np

import concourse.bass as bass
import concourse.tile as tile
from concourse import bacc, mybir
from concourse.bass_utils import run_bass_kernel_spmd

F32 = mybir.dt.float32
BF16 = mybir.dt.bfloat16
NP_BF16 = np.dtype(ml_dtypes.bfloat16)

B, C, H, W = 64, 256, 56, 56
HW = H * W  # 3136
N_CORES = 8
C_LOC = C // N_CORES  # 32 channels per core
CBLK = 8  # channels per block
N_BLOCKS = C_LOC // CBLK  # 4 blocks per core
BL = 128 // CBLK  # 16 b_lo values packed in the partition dim
BH = B // BL  # 4 quarter-tiles per block (b_hi)
N_TILE = N_BLOCKS * BH  # 16 tiles per core
SUB = 448  # bn_stats subgroup / PE chunk width (3136 = 7*448, <= 512)
NSUB = HW // SUB  # 7
HALF_N = 2 * HW  # elems per partition in either stat half = 6272
EPS = 1e-5

_NC_CACHE = {}


def _build_nc(nbufs=16):
    # Bacc (not plain Bass): its finalize() runs generate_event_semaphores,
    # which splits multi-sem waits — TRN2 instructions carry at most one.
    nc = bacc.Bacc()
    x = nc.dram_tensor("x", [N_TILE, 128, HW], BF16, kind="ExternalInput")
    y = nc.dram_tensor("y", [N_TILE, 128, HW], BF16, kind="ExternalOutput")
    gamma = nc.dram_tensor("gamma", [CBLK, N_BLOCKS], F32, kind="ExternalInput")
    beta = nc.dram_tensor("beta", [CBLK, N_BLOCKS], F32, kind="ExternalInput")
    sel8b = nc.dram_tensor("sel8b", [128, CBLK], BF16, kind="ExternalInput")
    sel8f = nc.dram_tensor("sel8f", [128, CBLK], F32, kind="ExternalInput")
    selT = nc.dram_tensor("selT", [CBLK, 128], F32, kind="ExternalInput")

    AF = mybir.ActivationFunctionType
    OP = mybir.AluOpType

    with ExitStack() as ctx:
        tc = ctx.enter_context(tile.TileContext(nc))
        xpool = ctx.enter_context(tc.tile_pool(name="xdata", bufs=nbufs))
        qpool = ctx.enter_context(tc.tile_pool(name="sqscr", bufs=2))
        spool = ctx.enter_context(tc.tile_pool(name="stats", bufs=4))
        cpool = ctx.enter_context(tc.tile_pool(name="const", bufs=1))
        ppool = ctx.enter_context(tc.tile_pool(name="psum", bufs=2, space="PSUM"))

        sel8b_t = cpool.tile([128, CBLK], BF16)
        nc.gpsimd.dma_start(out=sel8b_t, in_=sel8b[:, :])
        sel8f_t = cpool.tile([128, CBLK], F32)
        nc.gpsimd.dma_start(out=sel8f_t, in_=sel8f[:, :])
        selT_t = cpool.tile([CBLK, 128], F32)
        nc.gpsimd.dma_start(out=selT_t, in_=selT[:, :])
        gam_t = cpool.tile([CBLK, N_BLOCKS], F32)
        nc.gpsimd.dma_start(out=gam_t, in_=gamma[:, :])
        bet_t = cpool.tile([CBLK, N_BLOCKS], F32)
        nc.gpsimd.dma_start(out=bet_t, in_=beta[:, :])
        eps_t = cpool.tile([CBLK, 1], F32)
        nc.vector.memset(eps_t, EPS)

        def stats_phase_a(blk):
            """Quarters 0,1: loads + ACT sum(x^2) + PE per-channel sum/16."""
            xts = []
            pack = spool.tile([128, 4], F32)
            psum_s = ppool.tile([CBLK, SUB], F32, tag="ps")
            for q in range(2):
                xt = xpool.tile([128, HW], BF16, tag="x")
                nc.sync.dma_start(out=xt, in_=x[blk * BH + q, :, :])
                xts.append(xt)
                scr = qpool.tile([128, HW], BF16, tag="scr")
                nc.scalar.activation(
                    scr, xt, AF.Square, accum_out=pack[:, 2 + q : 3 + q]
                )
                xv = xt.rearrange("p (s f) -> p s f", f=SUB)
                for s in range(NSUB):
                    nc.tensor.matmul(
                        psum_s,
                        sel8b_t,
                        xv[:, s, :],
                        start=(q == 0 and s == 0),
                        stop=(q == 1 and s == NSUB - 1),
                    )
            return xts, pack, psum_s

        def stats_phase_b(blk, xts, pack, psum_s):
            """Quarters 2,3: loads + DVE bn_stats; then per-channel reduce."""
            stats = spool.tile([128, 2, NSUB, 6], F32)
            for q in range(2, 4):
                xt = xpool.tile([128, HW], BF16, tag="x")
                nc.sync.dma_start(out=xt, in_=x[blk * BH + q, :, :])
                xts.append(xt)
                xv = xt.rearrange("p (s f) -> p s f", f=SUB)
                for s in range(NSUB):
                    nc.vector.bn_stats(out=stats[:, q - 2, s, :], in_=xv[:, s, :])

            # DVE: per-partition [mean, var] over q2+q3 -> [mean, E[x^2]]
            nc.vector.bn_aggr(out=pack[:, 0:2], in_=stats)
            m2 = spool.tile([128, 1], F32)
            nc.vector.tensor_mul(m2, pack[:, 0:1], pack[:, 0:1])
            nc.vector.tensor_add(pack[:, 1:2], pack[:, 1:2], m2)

            # PE: per-channel [mean_q23, E2_q23, sumsq_q0, sumsq_q1] / 16
            pq = ppool.tile([CBLK, 4], F32, tag="pq")
            nc.tensor.matmul(pq, sel8f_t, pack, start=True, stop=True)
            return xts, psum_s, pq

        def norm_phase(blk, xts, psum_s, pq):
            """Fold + scalar tail + normalize + stores. Emitted one block
            late so the cross-engine round-trips hide under the next
            block's streaming work."""
            last = blk == N_BLOCKS - 1
            # fold PE sums: s4 = sum(x_q01)/16 per channel
            s4 = spool.tile([CBLK, 1], F32)
            nc.vector.reduce_sum(s4, psum_s, axis=mybir.AxisListType.X)
            mh1 = spool.tile([CBLK, 1], F32)
            nc.vector.tensor_scalar_mul(mh1, s4, 1.0 / HALF_N)
            # mean = (mean_q23 + mean_q01)/2
            mean = spool.tile([CBLK, 1], F32)
            nc.vector.tensor_scalar(
                out=mean, in0=pq[:, 0:1], scalar1=mh1, scalar2=0.5,
                op0=OP.add, op1=OP.mult,
            )
            # E[x^2] = (E2_q23 + (sumsq_q0 + sumsq_q1)/6272)/2
            e1 = spool.tile([CBLK, 1], F32)
            nc.vector.tensor_scalar_mul(e1, pq[:, 2:3], 1.0 / HALF_N)
            e2 = spool.tile([CBLK, 1], F32)
            nc.vector.tensor_scalar(
                out=e2, in0=pq[:, 3:4], scalar1=1.0 / HALF_N, scalar2=e1,
                op0=OP.mult, op1=OP.add,
            )
            ex2 = spool.tile([CBLK, 1], F32)
            nc.vector.tensor_scalar(
                out=ex2, in0=pq[:, 1:2], scalar1=e2, scalar2=0.5,
                op0=OP.add, op1=OP.mult,
            )
            m2b = spool.tile([CBLK, 1], F32)
            nc.vector.tensor_mul(m2b, mean, mean)
            var = spool.tile([CBLK, 1], F32)
            nc.vector.tensor_sub(var, ex2, m2b)
            std = spool.tile([CBLK, 1], F32)
            nc.scalar.activation(std, var, AF.Sqrt, bias=eps_t)
            rstd = spool.tile([CBLK, 1], F32)
            nc.vector.reciprocal(rstd, std)
            # A = gamma*rstd, B = beta - mean*A
            ab8 = spool.tile([CBLK, 2], F32)
            nc.vector.tensor_mul(ab8[:, 0:1], rstd, gam_t[:, blk : blk + 1])
            t4 = spool.tile([CBLK, 1], F32)
            nc.vector.tensor_mul(t4, mean, ab8[:, 0:1])
            nc.vector.tensor_sub(ab8[:, 1:2], bet_t[:, blk : blk + 1], t4)

            # broadcast (A, B) to all 128 partitions via PE matmul
            ps2 = ppool.tile([128, 2], F32, tag="pb")
            nc.tensor.matmul(ps2, selT_t, ab8, start=True, stop=True)
            ab = spool.tile([128, 2], F32)
            nc.vector.tensor_copy(ab, ps2)

            # normalize + store; stores ride the ACT HWDGE ring, pushed
            # right after the ACT norms so the waits barely stall ACT
            def norm_dve(q):
                nc.vector.tensor_scalar(
                    out=xts[q], in0=xts[q], scalar1=ab[:, 0:1],
                    scalar2=ab[:, 1:2], op0=OP.mult, op1=OP.add,
                )

            if last:
                for q in range(4):
                    norm_dve(q)
                    nc.scalar.dma_start(out=y[blk * BH + q, :, :], in_=xts[q])
            else:
                for q in range(2):
                    norm_dve(q + 2)
                for q in range(2):
                    nc.scalar.activation(
                        xts[q], xts[q], AF.Identity,
                        bias=ab[:, 1:2], scale=ab[:, 0:1],
                    )
                    nc.scalar.dma_start(out=y[blk * BH + q, :, :], in_=xts[q])
                for q in range(2, 4):
                    nc.scalar.dma_start(out=y[blk * BH + q, :, :], in_=xts[q])

        # One-block-deep software pipeline over the emission order; the
        # deferred tail of block k is emitted BETWEEN the two stat halves
        # of block k+1, so on the in-order DVE stream the chain+norms of
        # block k run while ACT/PE stream block k+1's first quarters —
        # never queued behind bn_stats that wait on late loads.
        prev = None
        for blk in range(N_BLOCKS):
            a = stats_phase_a(blk)
            if blk == 0:
                cur = stats_phase_b(blk, *a)
                norm_phase(blk, *cur)
                prev = None
            else:
                if prev is not None:
                    norm_phase(prev[0], *prev[1])
                cur = stats_phase_b(blk, *a)
                prev = (blk, cur)
        if prev is not None:
            norm_phase(prev[0], *prev[1])
    nc.finalize()
    return nc


def get_nc(nbufs=16):
    if nbufs not in _NC_CACHE:
        _NC_CACHE[nbufs] = _build_nc(nbufs)
    return _NC_CACHE[nbufs]


def _sel_matrices():
    # the 1/16 channel-indicator: reduce-matmuls on per-partition values
    # yield (sum over the channel's 16 partitions)/16
    sel = np.zeros((128, CBLK), dtype=np.float32)
    sel[np.arange(128), np.arange(128) % CBLK] = 1.0 / BL
    selT = np.zeros((CBLK, 128), dtype=np.float32)
    selT[np.arange(128) % CBLK, np.arange(128)] = 1.0
    return sel, selT


def pack_inputs(x, gamma, beta):
    """Full f32 inputs -> list of per-core in_maps (bf16 device layout)."""
    x16 = np.asarray(x, dtype=np.float32).astype(NP_BF16)
    gamma = np.asarray(gamma, dtype=np.float32)
    beta = np.asarray(beta, dtype=np.float32)
    # [b_hi, b_lo, core, blk, cc, hw] -> [core, blk, b_hi, b_lo, cc, hw]
    xr = np.ascontiguousarray(
        x16.reshape(BH, BL, N_CORES, N_BLOCKS, CBLK, HW).transpose(2, 3, 0, 1, 4, 5)
    )
    g = gamma.reshape(N_CORES, N_BLOCKS, CBLK)
    bt = beta.reshape(N_CORES, N_BLOCKS, CBLK)
    sel, selT = _sel_matrices()
    sel8b = sel.astype(NP_BF16)  # 1/16 is exact in bf16
    in_maps = []
    for i in range(N_CORES):
        in_maps.append(
            {
                "x": xr[i].reshape(N_TILE, 128, HW),
                "gamma": np.ascontiguousarray(g[i].T),
                "beta": np.ascontiguousarray(bt[i].T),
                "sel8b": sel8b,
                "sel8f": sel,
                "selT": selT,
            }
        )
    return in_maps


def unpack_outputs(per_core_y):
    """List of per-core y (bf16 device layout) -> full f32 (64,256,56,56)."""
    ys = np.stack(per_core_y)  # [core, blk*b_hi, 128, hw] bf16
    out = (
        ys.reshape(N_CORES, N_BLOCKS, BH, BL, CBLK, HW)
        .transpose(2, 3, 0, 1, 4, 5)
        .astype(np.float32)
        .reshape(B, C, H, W)
    )
    return out


def run(inputs, trace=False, nbufs=16):
    """Returns (full_output, BassKernelResults)."""
    nc = get_nc(nbufs)
    in_maps = pack_inputs(inputs["x"], inputs["gamma"], inputs["beta"])
    res = run_bass_kernel_spmd(nc, in_maps, list(range(N_CORES)), trace=trace)
    out = unpack_outputs([r["y"] for r in res.results])
    return out, res


def kernel(**inputs):
    out, _ = run(inputs)
    return out


# revision 13
# speedup vs baseline: 1.2291x; 1.1529x over previous
"""Training-mode BatchNorm2d over x(64,256,56,56) f32 on 8 trn2 NeuronCores.

Sharding: channel-parallel (32 channels per core) — each core owns complete
per-channel reductions, so no cross-core collectives are needed.

The 2e-2 rel-err budget admits a bf16 HBM data path: the host converts x to
bf16 (max rounding error ~2^-9 of value), the device reads bf16, computes
stats in f32, normalizes, and writes bf16 back; the host converts the output
to f32. HBM traffic per core halves to 12.85 MB read + 12.85 MB write
(~63us at the measured per-core aggregate DMA rate) — the floor this kernel
is built around.

Layout: per core 8 channel-blocks of 4 channels; each block is two
half-tiles [128p, 3136] bf16 (partition p = b_lo*4 + cc, half = b_hi), so
16 loads + 16 stores of 800KB. All 16 halves stay resident in SBUF (12.25
MB) between the stats pass and the normalize pass (minimal 2x HBM traffic).

Stats are spread so every engine stays well under the ~7.75us/block DMA
pace (the exact mean and exact variance are both computed — no
approximation beyond the bf16 rounding):
 - per-channel sum(x) for BOTH halves on the (otherwise idle)
   TensorEngine: 7 matmuls per half of x-chunks [128, 448] (moving, bf16)
   against a (1/32)-scaled channel-indicator (stationary, bf16; 1/32 is
   exact), PSUM-accumulated into [4, 448] and folded by one DVE
   reduce_sum.
 - per-partition sum(x^2): half 0 via ScalarE Square activation with
   accum_out (~3.4us); half 1 via DVE tensor_tensor_reduce x*x with
   accum_out, which runs in 2x bf16 mode (~1.7us). Both accumulate f32.
 - the two per-partition sumsq columns are reduced per-channel by one
   tiny f32 matmul.
 - scalar tail (var, rsqrt, A=gamma*rstd, B=beta-mean*A) on DVE (its small
   ops are ~3x cheaper than ACT's); Sqrt on ACT (DVE has none); (A, B)
   broadcast to 128 partitions by a tiny PE matmul.
 - normalize x*A + B in place on DVE for both halves (tensor_scalar, 4x
   bf16 mode, ~1.25us per half).

Per-block engine busy vs the 7.75us DMA window: DVE ~6.1us, ACT ~4.0us,
PE ~6.0us — the DMA stream is the pacer throughout.

The tail of block k is emitted between block k+1's two stat halves, so on
the in-order DVE stream the chain+norms of block k run while block k+1's
half-1 load is still in flight, and the half-1 square starts the moment
the load lands. Input DMAs ride the SP HWDGE ring (no waits ever land
there, so all 16 loads stream back-to-back); output DMAs ride the ACT
HWDGE ring, pushed right after the DVE norms complete.
"""

from contextlib import ExitStack

import ml_dtypes
import numpy as np

import concourse.bass as bass
import concourse.tile as tile
from concourse import bacc, mybir
from concourse.bass_utils import run_bass_kernel_spmd

F32 = mybir.dt.float32
BF16 = mybir.dt.bfloat16
NP_BF16 = np.dtype(ml_dtypes.bfloat16)

B, C, H, W = 64, 256, 56, 56
HW = H * W  # 3136
N_CORES = 8
C_LOC = C // N_CORES  # 32 channels per core
CBLK = 4  # channels per block
N_BLOCKS = C_LOC // CBLK  # 8 blocks per core
BL = 128 // CBLK  # 32 b_lo values packed in the partition dim
BH = B // BL  # 2 half-tiles per block (b_hi)
N_TILE = N_BLOCKS * BH  # 16 tiles per core
SUB = 448  # PE sum-matmul chunk width (3136 = 7*448, <= 512 PSUM cols)
NSUB = HW // SUB  # 7
N_PART = BH * HW  # elems per partition per block = 6272
EPS = 1e-5

_NC_CACHE = {}


def _build_nc(nbufs=16):
    # Bacc (not plain Bass): its finalize() runs generate_event_semaphores,
    # which splits multi-sem waits — TRN2 instructions carry at most one.
    nc = bacc.Bacc()
    x = nc.dram_tensor("x", [N_TILE, 128, HW], BF16, kind="ExternalInput")
    y = nc.dram_tensor("y", [N_TILE, 128, HW], BF16, kind="ExternalOutput")
    gamma = nc.dram_tensor("gamma", [CBLK, N_BLOCKS], F32, kind="ExternalInput")
    beta = nc.dram_tensor("beta", [CBLK, N_BLOCKS], F32, kind="ExternalInput")
    sel8b = nc.dram_tensor("sel8b", [128, CBLK], BF16, kind="ExternalInput")
    sel8f = nc.dram_tensor("sel8f", [128, CBLK], F32, kind="ExternalInput")
    selT = nc.dram_tensor("selT", [CBLK, 128], F32, kind="ExternalInput")

    AF = mybir.ActivationFunctionType
    OP = mybir.AluOpType

    with ExitStack() as ctx:
        tc = ctx.enter_context(tile.TileContext(nc))
        xpool = ctx.enter_context(tc.tile_pool(name="xdata", bufs=nbufs))
        qpool = ctx.enter_context(tc.tile_pool(name="sqscr", bufs=4))
        spool = ctx.enter_context(tc.tile_pool(name="stats", bufs=4))
        cpool = ctx.enter_context(tc.tile_pool(name="const", bufs=1))
        ppool = ctx.enter_context(tc.tile_pool(name="psum", bufs=2, space="PSUM"))

        sel8b_t = cpool.tile([128, CBLK], BF16)
        nc.gpsimd.dma_start(out=sel8b_t, in_=sel8b[:, :])
        sel8f_t = cpool.tile([128, CBLK], F32)
        nc.gpsimd.dma_start(out=sel8f_t, in_=sel8f[:, :])
        selT_t = cpool.tile([CBLK, 128], F32)
        nc.gpsimd.dma_start(out=selT_t, in_=selT[:, :])
        gam_t = cpool.tile([CBLK, N_BLOCKS], F32)
        nc.gpsimd.dma_start(out=gam_t, in_=gamma[:, :])
        bet_t = cpool.tile([CBLK, N_BLOCKS], F32)
        nc.gpsimd.dma_start(out=bet_t, in_=beta[:, :])
        eps_t = cpool.tile([CBLK, 1], F32)
        nc.vector.memset(eps_t, EPS)

        def sum_mms(psum_s, xt, j):
            xv = xt.rearrange("p (s f) -> p s f", f=SUB)
            for s in range(NSUB):
                nc.tensor.matmul(
                    psum_s,
                    sel8b_t,
                    xv[:, s, :],
                    start=(j == 0 and s == 0),
                    stop=(j == 1 and s == NSUB - 1),
                )

        def stats_phase_a(blk):
            """Half 0: load + ACT sum(x^2) + PE sum chunks."""
            pack = spool.tile([128, 2], F32)
            psum_s = ppool.tile([CBLK, SUB], F32, tag="ps")
            xt0 = xpool.tile([128, HW], BF16, tag="x")
            nc.sync.dma_start(out=xt0, in_=x[blk * BH, :, :])
            scr = qpool.tile([128, HW], BF16, tag="scra")
            nc.scalar.activation(scr, xt0, AF.Square, accum_out=pack[:, 0:1])
            sum_mms(psum_s, xt0, 0)
            return xt0, pack, psum_s

        def stats_phase_b(blk, xt0, pack, psum_s):
            """Half 1: load + DVE sum(x^2) + PE sum chunks + reduce."""
            xt1 = xpool.tile([128, HW], BF16, tag="x")
            nc.sync.dma_start(out=xt1, in_=x[blk * BH + 1, :, :])
            scr = qpool.tile([128, HW], BF16, tag="scrv")
            nc.vector.scalar_tensor_tensor(
                out=scr,
                in0=xt1,
                scalar=1.0,
                in1=xt1,
                op0=OP.mult,
                op1=OP.mult,
                accum_out=pack[:, 1:2],
            )
            sum_mms(psum_s, xt1, 1)
            # PE: per-channel [sumsq_h0, sumsq_h1] / 32
            pq = ppool.tile([CBLK, 2], F32, tag="pq")
            nc.tensor.matmul(pq, sel8f_t, pack, start=True, stop=True)
            return xt0, xt1, psum_s, pq

        def norm_phase(blk, xt0, xt1, psum_s, pq):
            """Fold + scalar tail + normalize + stores. Emitted between the
            next block's two stat halves so the cross-engine round-trips
            hide under its streaming work."""
            # fold PE sums: mean = (sum/32)/6272 per channel
            s4 = spool.tile([CBLK, 1], F32)
            nc.vector.reduce_sum(s4, psum_s, axis=mybir.AxisListType.X)
            mean = spool.tile([CBLK, 1], F32)
            nc.vector.tensor_scalar_mul(mean, s4, 1.0 / N_PART)
            # E[x^2] = (sumsq_h0 + sumsq_h1)/32/6272
            e1 = spool.tile([CBLK, 1], F32)
            nc.vector.tensor_scalar_mul(e1, pq[:, 0:1], 1.0 / N_PART)
            ex2 = spool.tile([CBLK, 1], F32)
            nc.vector.tensor_scalar(
                out=ex2, in0=pq[:, 1:2], scalar1=1.0 / N_PART, scalar2=e1,
                op0=OP.mult, op1=OP.add,
            )
            m2b = spool.tile([CBLK, 1], F32)
            nc.vector.tensor_mul(m2b, mean, mean)
            var = spool.tile([CBLK, 1], F32)
            nc.vector.tensor_sub(var, ex2, m2b)
            std = spool.tile([CBLK, 1], F32)
            nc.scalar.activation(std, var, AF.Sqrt, bias=eps_t)
            rstd = spool.tile([CBLK, 1], F32)
            nc.vector.reciprocal(rstd, std)
            # A = gamma*rstd, B = beta - mean*A
            ab8 = spool.tile([CBLK, 2], F32)
            nc.vector.tensor_mul(ab8[:, 0:1], rstd, gam_t[:, blk : blk + 1])
            t4 = spool.tile([CBLK, 1], F32)
            nc.vector.tensor_mul(t4, mean, ab8[:, 0:1])
            nc.vector.tensor_sub(ab8[:, 1:2], bet_t[:, blk : blk + 1], t4)

            # broadcast (A, B) to all 128 partitions via PE matmul
            ps2 = ppool.tile([128, 2], F32, tag="pb")
            nc.tensor.matmul(ps2, selT_t, ab8, start=True, stop=True)
            ab = spool.tile([128, 2], F32)
            nc.vector.tensor_copy(ab, ps2)

            # normalize both halves on DVE; stores ride the ACT HWDGE ring
            for q, xt in ((0, xt0), (1, xt1)):
                nc.vector.tensor_scalar(
                    out=xt, in0=xt, scalar1=ab[:, 0:1], scalar2=ab[:, 1:2],
                    op0=OP.mult, op1=OP.add,
                )
                nc.scalar.dma_start(out=y[blk * BH + q, :, :], in_=xt)

        # Software pipeline: the tail of block k is emitted between block
        # k+1's two stat halves, so the DVE chain runs while k+1's half-1
        # load is in flight and ttr starts the moment that load lands.
        prev = None
        for blk in range(N_BLOCKS):
            a = stats_phase_a(blk)
            if blk == 0:
                cur = stats_phase_b(blk, *a)
                norm_phase(blk, *cur)
                prev = None
            else:
                if prev is not None:
                    norm_phase(prev[0], *prev[1])
                cur = stats_phase_b(blk, *a)
                prev = (blk, cur)
        if prev is not None:
            norm_phase(prev[0], *prev[1])
    nc.finalize()
    return nc


def get_nc(nbufs=16):
    if nbufs not in _NC_CACHE:
        _NC_CACHE[nbufs] = _build_nc(nbufs)
    return _NC_CACHE[nbufs]


def _sel_matrices():
    # the 1/32 channel-indicator: reduce-matmuls on per-partition values
    # yield (sum over the channel's 32 partitions)/32
    sel = np.zeros((128, CBLK), dtype=np.float32)
    sel[np.arange(128), np.arange(128) % CBLK] = 1.0 / BL
    selT = np.zeros((CBLK, 128), dtype=np.float32)
    selT[np.arange(128) % CBLK, np.arange(128)] = 1.0
    return sel, selT


def pack_inputs(x, gamma, beta):
    """Full f32 inputs -> list of per-core in_maps (bf16 device layout)."""
    x16 = np.asarray(x, dtype=np.float32).astype(NP_BF16)
    gamma = np.asarray(gamma, dtype=np.float32)
    beta = np.asarray(beta, dtype=np.float32)
    # [b_hi, b_lo, core, blk, cc, hw] -> [core, blk, b_hi, b_lo, cc, hw]
    xr = np.ascontiguousarray(
        x16.reshape(BH, BL, N_CORES, N_BLOCKS, CBLK, HW).transpose(2, 3, 0, 1, 4, 5)
    )
    g = gamma.reshape(N_CORES, N_BLOCKS, CBLK)
    bt = beta.reshape(N_CORES, N_BLOCKS, CBLK)
    sel, selT = _sel_matrices()
    sel8b = sel.astype(NP_BF16)  # 1/32 is exact in bf16
    in_maps = []
    for i in range(N_CORES):
        in_maps.append(
            {
                "x": xr[i].reshape(N_TILE, 128, HW),
                "gamma": np.ascontiguousarray(g[i].T),
                "beta": np.ascontiguousarray(bt[i].T),
                "sel8b": sel8b,
                "sel8f": sel,
                "selT": selT,
            }
        )
    return in_maps


def unpack_outputs(per_core_y):
    """List of per-core y (bf16 device layout) -> full f32 (64,256,56,56)."""
    ys = np.stack(per_core_y)  # [core, blk*b_hi, 128, hw] bf16
    out = (
        ys.reshape(N_CORES, N_BLOCKS, BH, BL, CBLK, HW)
        .transpose(2, 3, 0, 1, 4, 5)
        .astype(np.float32)
        .reshape(B, C, H, W)
    )
    return out


def run(inputs, trace=False, nbufs=16):
    """Returns (full_output, BassKernelResults)."""
    nc = get_nc(nbufs)
    in_maps = pack_inputs(inputs["x"], inputs["gamma"], inputs["beta"])
    res = run_bass_kernel_spmd(nc, in_maps, list(range(N_CORES)), trace=trace)
    out = unpack_outputs([r["y"] for r in res.results])
    return out, res


def kernel(**inputs):
    out, _ = run(inputs)
    return out
